# revision 1
# baseline (speedup 1.0000x reference)
"""CrossNetwork (DCN) forward on 8 TRN2 NeuronCores.

Reference computation (per cross layer i, x0 = input):
    s_i = xl . w_i            (per-row scalar)
    xl  = x0 * s_i + b_i + xl

Algebraic collapse: xl_i = alpha_i * x0 + c_i with per-row scalar alpha_i
and a row-constant vector c_i = sum_{j<i} b_j. Hence:
    u_i       = x0 . w_i                      (3 dots per row, all vs x0)
    alpha_0   = 1,  alpha_{i+1} = alpha_i * (1 + u_i) + (c_i . w_i)
    out       = alpha_3 * x0 + c_3
One read of x, one write of out -> memory roofline (~33.5 MB/core, ~88 us
at the ~380 GB/s/core the DMA fabric delivers).

Sharding: pure data parallel over the batch dim, weights replicated.

Zero-b fast path (the reference always passes b = 0): out = alpha3 * x
with alpha3 = (1+u0)(1+u1)(1+u2). Measured-on-silicon design rules:
  - each engine gets exactly one role so no store wait can stall a
    producer: Pool issues casting loads (x f32 in DRAM -> bf16 in SBUF,
    SWDGE is the only DGE that casts), DVE runs the three dot products
    (scalar_tensor_tensor with f32 accum; bf16 halves SBUF pressure),
    ACT runs the 3-op alpha recurrence (mul-add fused into activation
    scale+bias) plus the final scale (bf16 -> f32), and SP issues all
    stores.
  - cross-engine producer->consumer chains through Pool or extra ACT
    instructions cost far more on hardware than the cost model predicts;
    keeping instruction count minimal beats balancing engine work.
bf16 values are well inside the 2e-2 tolerance (measured ~5e-3).

The general-b path keeps full f32 precision and the bias constants.

reps > 1 repeats the main loop in-NEFF (benchmarking only).
"""

import contextlib

import numpy as np

import concourse.bacc as bacc
import concourse.mybir as mybir
import concourse.tile as tile
from concourse.bass_utils import run_bass_kernel_spmd

N_CORES = 8
B, D, CROSS = 16384, 2048, 3
P = 128
F32 = mybir.dt.float32
BF16 = mybir.dt.bfloat16


def build_body_zero_b(tc, x_ap, w_ap, b_ap, out_ap, rows, reps=1):
    nc = tc.nc
    nt = rows // P
    Al = mybir.AluOpType
    Act = mybir.ActivationFunctionType

    with contextlib.ExitStack() as ctx:
        const = ctx.enter_context(tc.tile_pool(name="const", bufs=1))
        xbpool = ctx.enter_context(tc.tile_pool(name="xb", bufs=8))
        ypool = ctx.enter_context(tc.tile_pool(name="y", bufs=6))
        spool = ctx.enter_context(tc.tile_pool(name="scr", bufs=2))
        upool = ctx.enter_context(tc.tile_pool(name="u", bufs=24))

        # w_i rows broadcast across partitions and cast to bf16 in one
        # stride-0 SWDGE read each.
        wb = []
        for i in range(CROSS):
            wbi = const.tile([P, D], BF16, tag=f"wb{i}")
            nc.gpsimd.dma_start(out=wbi[:],
                                in_=w_ap[i : i + 1, :].to_broadcast([P, D]))
            wb.append(wbi)

        for i in range(nt * reps):
            t = i % nt
            xb = xbpool.tile([P, D], BF16, tag="xb")
            nc.gpsimd.dma_start(out=xb[:], in_=x_ap[t * P : (t + 1) * P, :])

            us = []
            for j in range(CROSS):
                u = upool.tile([P, 1], F32, tag=f"u{j}")
                scr = spool.tile([P, D], BF16, tag="scr")
                nc.vector.scalar_tensor_tensor(
                    out=scr[:], in0=xb[:], scalar=0.0, in1=wb[j][:],
                    op0=Al.bypass, op1=Al.mult, accum_out=u[:])
                us.append(u)

            # alpha3 = (1+u0)(1+u1)(1+u2) in 3 ACT ops:
            # t1 = 1+u0; a2 = t1*u1 + t1; a3 = a2*u2 + a2
            t1 = upool.tile([P, 1], F32, tag="t1")
            nc.scalar.add(t1[:], us[0][:], 1.0)
            a2 = upool.tile([P, 1], F32, tag="a2")
            nc.scalar.activation(a2[:], us[1][:], Act.Identity, bias=t1[:],
                                 scale=t1[:])
            a3 = upool.tile([P, 1], F32, tag="a3")
            nc.scalar.activation(a3[:], us[2][:], Act.Identity, bias=a2[:],
                                 scale=a2[:])

            yt = ypool.tile([P, D], F32, tag="y")
            nc.scalar.activation(yt[:], xb[:], Act.Copy, scale=a3[:])
            nc.sync.dma_start(out=out_ap[t * P : (t + 1) * P, :], in_=yt[:])


def build_body_general(tc, x_ap, w_ap, b_ap, out_ap, rows):
    """General-b path: full f32, bias constants, ACT scale + Pool bias-add."""
    nc = tc.nc
    nt = rows // P
    Al = mybir.AluOpType
    Act = mybir.ActivationFunctionType

    with contextlib.ExitStack() as ctx:
        const = ctx.enter_context(tc.tile_pool(name="const", bufs=1))
        xpool = ctx.enter_context(tc.tile_pool(name="x", bufs=4))
        ypool = ctx.enter_context(tc.tile_pool(name="y", bufs=4))
        spool = ctx.enter_context(tc.tile_pool(name="scr", bufs=3))
        upool = ctx.enter_context(tc.tile_pool(name="u", bufs=16))

        # Load each tiny w_i / b_i row to partition 0, then replicate across
        # all 128 partitions on-chip (gpsimd partition_broadcast). The custom
        # op requires its input AP to start at partition 0, hence one [1, D]
        # tile per row. All row tiles are transient (pre pool).
        with tc.tile_pool(name="pre", bufs=1) as pre:
            wrow = []
            brow = []
            for i in range(CROSS):
                wr = pre.tile([1, D], F32, tag=f"wr{i}")
                nc.sync.dma_start(out=wr[:], in_=w_ap[i : i + 1, :])
                wrow.append(wr)
                br = pre.tile([1, D], F32, tag=f"br{i}")
                nc.sync.dma_start(out=br[:], in_=b_ap[i : i + 1, :])
                brow.append(br)

            wbc = []
            for i in range(CROSS):
                wt = const.tile([P, D], F32, tag=f"w{i}")
                nc.gpsimd.partition_broadcast(wt[:], wrow[i][:])
                wbc.append(wt)

            # row constants on [1, D]: c2 = b0 + b1, c3 = c2 + b2
            c2row = pre.tile([1, D], F32, tag="c2r")
            nc.vector.tensor_add(c2row[:], brow[0][:], brow[1][:])
            c3row = pre.tile([1, D], F32, tag="c3r")
            nc.vector.tensor_add(c3row[:], c2row[:], brow[2][:])
            c3bc = const.tile([P, D], F32, tag="c3")
            nc.gpsimd.partition_broadcast(c3bc[:], c3row[:])

            # k1 = b0 . w1, k2 = c2 . w2 (scalars), then replicate to [P, 1]
            k1row = pre.tile([1, 1], F32, tag="k1r")
            scr_k1 = pre.tile([1, D], F32, tag="scrr")
            nc.vector.scalar_tensor_tensor(
                out=scr_k1[:], in0=brow[0][:], scalar=0.0, in1=wrow[1][:],
                op0=Al.bypass, op1=Al.mult, accum_out=k1row[:],
            )
            k2row = pre.tile([1, 1], F32, tag="k2r")
            scr_k2 = pre.tile([1, D], F32, tag="scrr2")
            nc.vector.scalar_tensor_tensor(
                out=scr_k2[:], in0=c2row[:], scalar=0.0, in1=wrow[2][:],
                op0=Al.bypass, op1=Al.mult, accum_out=k2row[:],
            )
            k1bc = const.tile([P, 1], F32, tag="k1")
            nc.gpsimd.partition_broadcast(k1bc[:], k1row[:])
            k2bc = const.tile([P, 1], F32, tag="k2")
            nc.gpsimd.partition_broadcast(k2bc[:], k2row[:])

        for t in range(nt):
            xt = xpool.tile([P, D], F32, tag="x")
            nc.sync.dma_start(out=xt[:], in_=x_ap[t * P : (t + 1) * P, :])

            us = []
            for i in range(CROSS):
                u = upool.tile([P, 1], F32, tag=f"u{i}")
                scr = spool.tile([P, D], F32, tag="scr")
                nc.vector.scalar_tensor_tensor(
                    out=scr[:], in0=xt[:], scalar=0.0, in1=wbc[i][:],
                    op0=Al.bypass, op1=Al.mult, accum_out=u[:],
                )
                us.append(u)

            # alpha recurrence on ACT: a3 = ((1+u0)(1+u1) + k1)(1+u2) + k2
            t1 = upool.tile([P, 1], F32, tag="t1")
            nc.scalar.add(t1[:], us[0][:], 1.0)
            t2 = upool.tile([P, 1], F32, tag="t2")
            nc.scalar.add(t2[:], us[1][:], 1.0)
            a2 = upool.tile([P, 1], F32, tag="a2")
            nc.scalar.activation(a2[:], t2[:], Act.Identity, bias=k1bc[:], scale=t1[:])
            t3 = upool.tile([P, 1], F32, tag="t3")
            nc.scalar.add(t3[:], us[2][:], 1.0)
            a3 = upool.tile([P, 1], F32, tag="a3")
            nc.scalar.activation(a3[:], t3[:], Act.Identity, bias=k2bc[:], scale=a2[:])

            # out = alpha3 * x0 + c3: scale on ACT, bias-add in place on Pool
            yt = ypool.tile([P, D], F32, tag="y")
            nc.scalar.activation(yt[:], xt[:], Act.Copy, scale=a3[:])
            nc.gpsimd.tensor_tensor(out=yt[:], in0=yt[:], in1=c3bc[:], op=Al.add)
            nc.sync.dma_start(out=out_ap[t * P : (t + 1) * P, :], in_=yt[:])


_CACHE = {}


def get_nc(rows, zero_b=False, reps=1):
    key = (rows, zero_b, reps)
    if key not in _CACHE:
        nc = bacc.Bacc(
            "TRN2",
            target_bir_lowering=False,
            debug=False,
            enable_asserts=False,
            num_devices=N_CORES,
        )
        x = nc.dram_tensor("x", [rows, D], F32, kind="ExternalInput").ap()
        w = nc.dram_tensor("W", [CROSS, D], F32, kind="ExternalInput").ap()
        b = nc.dram_tensor("b", [CROSS, D], F32, kind="ExternalInput").ap()
        out = nc.dram_tensor("out", [rows, D], F32, kind="ExternalOutput").ap()
        with tile.TileContext(nc) as tc:
            if zero_b:
                build_body_zero_b(tc, x, w, b, out, rows, reps=reps)
            else:
                build_body_general(tc, x, w, b, out, rows)
        nc.compile()
        _CACHE[key] = nc
    return _CACHE[key]


def run(x, W, b, trace=False, force_general=False):
    x = np.ascontiguousarray(np.asarray(x, dtype=np.float32))
    W = np.ascontiguousarray(np.asarray(W, dtype=np.float32))
    b = np.ascontiguousarray(np.asarray(b, dtype=np.float32))
    rows = x.shape[0] // N_CORES
    zero_b = (not force_general) and not b.any()
    nc = get_nc(rows, zero_b)
    in_maps = [
        {"x": x[i * rows : (i + 1) * rows], "W": W, "b": b} for i in range(N_CORES)
    ]
    try:
        res = run_bass_kernel_spmd(
            nc, in_maps, core_ids=list(range(N_CORES)), trace=trace
        )
    except ModuleNotFoundError:
        # BASS_TRACE in the environment routes through an NTFF profile hook
        # that is absent in some containers; fall back to an untraced run.
        import os

        os.environ["BASS_NEVER_TRACE"] = "1"
        res = run_bass_kernel_spmd(
            nc, in_maps, core_ids=list(range(N_CORES)), trace=False
        )
    out = np.concatenate([r["out"] for r in res.results], axis=0)
    return out, res


def kernel(x, W, b):
    out, _ = run(x, W, b)
    return out

